# revision 18
# baseline (speedup 1.0000x reference)
"""Trainium2 Bass kernel for nn_CrossmotionModule (gnn_message_passing).

Reference computation (B=4, M=256, T=64, Dm=512, E=768):
    rel[b,m,t,n,k] = (c[b,m,t,k] - c[b,n,t,k]) * vis[b,m,t] * vis[b,n,t]
    fea[b,t,m,(n,k)] = rel                  # (B,T,M,512)
    h   = fea @ W1 + b1                     # (B,T,M,512)
    out = [h, pos] @ W2 + b2                # (B,T,M,768)

Algebraic collapse: with p = vis (BT,M), u0 = p*c0, u1 = p*c1 and the
host-folded fused weight V2 = W1 @ W2[:512] (512, 768):
    out[bt,m,e] = u0[m]*G0[e] + u1[m]*G1[e] - p[m]*G2[e] + const[m,e]
    G0[e] = sum_n p[n] V2[2n, e]
    G1[e] = sum_n p[n] V2[2n+1, e]
    G2[e] = sum_nk (p*c)[nk] V2[nk, e]
    const = b1 @ W2[:512] + pos @ W2[512:] + b2

The problem is HBM-write bound (192 MiB fp32 output) and, per-core, the
PE is output-streaming bound (the hardware throttles the PE to 1.2 GHz
under sustained load, ~0.83 ns per 128-lane PSUM column; measured 1280ns
per 1536-column row, back to back).  Design:

  * fp16 end-to-end (relative error ~4e-4, far under the 2e-2 gate):
    halves the output DMA traffic vs fp32 and needs no bf16-style split
    compensation anywhere.
  * const is rank-1 when pos == 0 (always true for this model): folded
    into the matmul as a 4th contraction row [u0, u1, -p, 1] x
    [G0; G1; G2; cvec] -- no per-row vector adds at all.  If pos != 0
    the (M,E) correction pos @ W2[512:] is added on the host.
  * G is computed on-device (L^T @ V2, fp32 PSUM), converted to fp16,
    and partition-regrouped (j*R+r, e) -> (j, r*E+e) through a DRAM
    bounce (matmul operands require contiguous 32-aligned partition
    ranges, so the regroup cannot be avoided; SBUF->SBUF regroup DMAs
    produce corrupt data on this toolchain).
  * vh arrives in 4 column chunks gating the 4 accumulation steps, so
    G matmuls start as soon as the first chunk lands.
  * Per row: 4 tiny-K (K=4) matmuls into two half-row PSUM tiles (2
    banks each, 2 generations = all 8 banks); fp32->fp16 PSUM->SBUF
    converts alternate DVE/Act; output DMAs drain a deep SBUF ring.

Sharding: data-parallel over bt = (b,t) flattened; 256 rows / 8 cores =
32 rows per core.  Weights replicated.  No cross-device communication.
"""

import numpy as np

B, M, T = 4, 256, 64
D_MOT, D_ABS, D_OUT = 512, 512, 768
N_CORES = 8
BT = B * T            # 256
R = BT // N_CORES     # 32 bt rows per core
E = D_OUT

_CACHED_NC = None


def _build_nc():
    """Build the SPMD Bass program (identical for all 8 cores)."""
    import concourse.bacc as bacc
    import concourse.bass as bass
    import concourse.mybir as mybir
    import concourse.tile as tile

    f32 = mybir.dt.float32
    f16 = mybir.dt.float16
    PSUM = bass.MemorySpace.PSUM

    nc = bacc.Bacc("TRN2", target_bir_lowering=False, debug=False)

    # Per-core inputs (host-prepared layouts; see _prep_inputs).
    la_d = nc.dram_tensor("la", [128, 4 * 96], f16, kind="ExternalInput")
    vh_d = nc.dram_tensor("vh", [128, 4 * E], f16, kind="ExternalInput")
    ut_d = nc.dram_tensor("ut", [4, R * 256], f16, kind="ExternalInput")
    cv_d = nc.dram_tensor("cv", [1, R * E], f16, kind="ExternalInput")
    out_d = nc.dram_tensor("out", [128, R * 2 * E], f16, kind="ExternalOutput")
    gscr_d = nc.dram_tensor("gscr", [3, R * E], f16)

    with tile.TileContext(nc) as tc:
        with tc.tile_pool(name="persist", bufs=1) as pers:
            ut_sb = pers.tile([4, R * 256], f16)
            g_sb = pers.tile([4, R * E], f16)

            # ---- prologue: G[(j,r), e] = L^T @ V2 ----
            with (
                tc.tile_pool(name="pro", bufs=1) as pro,
                tc.tile_pool(name="prop", bufs=1, space=PSUM) as prop,
            ):
                la_sb = pro.tile([128, 4 * 96], f16)
                vh_sb = pro.tile([128, 4 * E], f16)
                # la + vh chunk kk gate the kk-th accumulation step (region
                # deps); issue in critical-path order so later chunks arrive
                # while earlier accumulation steps run.
                nc.sync.dma_start(vh_sb[:, 0:E], vh_d[:, 0:E])
                nc.sync.dma_start(la_sb[:], la_d[:])
                for kk in range(1, 4):
                    nc.sync.dma_start(
                        vh_sb[:, kk * E : (kk + 1) * E],
                        vh_d[:, kk * E : (kk + 1) * E],
                    )
                nc.sync.dma_start(ut_sb[:], ut_d[:])
                nc.sync.dma_start(g_sb[3:4, :], cv_d[:])

                ghl_sb = pro.tile([96, E], f16)
                g_ps = prop.tile([96, E], f32)
                for kk in range(4):
                    for lo, hi in ((0, 512), (512, 768)):
                        nc.tensor.matmul(
                            g_ps[:, lo:hi],
                            la_sb[:, kk * 96 : (kk + 1) * 96],
                            vh_sb[:, kk * E + lo : kk * E + hi],
                            start=(kk == 0),
                            stop=(kk == 3),
                        )
                nc.scalar.copy(ghl_sb[:], g_ps[:])
                # Partition regroup (j*R+r, e)->(j, r*E+e) via DRAM bounce:
                # scatter on the write side, read back in 2 r-chunks so the
                # first main-loop matmuls only wait on chunk 0.
                nc.sync.dma_start(
                    gscr_d.rearrange("j (r e) -> (j r) e", r=R), ghl_sb[:]
                )
                CK = R // 2 * E
                for ck in range(2):
                    nc.sync.dma_start(
                        g_sb[0:3, ck * CK : (ck + 1) * CK],
                        gscr_d[:, ck * CK : (ck + 1) * CK],
                    )

            # ---- main loop: out[m, (r,w,e)] = U4_r^T G4_r ----
            groups = (
                [[0], [1]]
                + [list(range(g, g + 2)) for g in range(2, R - 4, 2)]
                + [[r] for r in range(R - 4, R)]
            )
            with (
                tc.tile_pool(name="mp", bufs=2, space=PSUM) as mp,
                tc.tile_pool(name="op", bufs=12) as op,
            ):
                ri = 0
                for grp in groups:
                    nq = len(grp)
                    out_sb = op.tile([128, nq * 1536], f16, tag="out_sb")
                    for q, r in enumerate(grp):
                        # Two half-row PSUM tiles (2 banks each) so the PE
                        # recycles banks at half-row grain and never stalls
                        # on a whole-row copy.
                        psL = mp.tile([128, 768], f32)
                        psR = mp.tile([128, 768], f32)
                        u0 = ut_sb[:, r * 256 : r * 256 + 128]
                        u1 = ut_sb[:, r * 256 + 128 : r * 256 + 256]
                        g0 = r * E
                        nc.tensor.matmul(psL[:, 0:512], u0, g_sb[:, g0 : g0 + 512])
                        nc.tensor.matmul(psL[:, 512:768], u0, g_sb[:, g0 + 512 : g0 + 768])
                        nc.tensor.matmul(psR[:, 0:512], u1, g_sb[:, g0 : g0 + 512])
                        nc.tensor.matmul(psR[:, 512:768], u1, g_sb[:, g0 + 512 : g0 + 768])
                        # GPSIMD cannot read PSUM; alternate DVE / Act.
                        for bi, ps in enumerate((psL, psR)):
                            dst = out_sb[
                                :, q * 1536 + bi * 768 : q * 1536 + (bi + 1) * 768
                            ]
                            if ri % 2 == 0:
                                nc.vector.tensor_copy(dst, ps[:])
                            else:
                                nc.scalar.copy(dst, ps[:])
                            ri += 1
                    nc.sync.dma_start(
                        out_d[:, grp[0] * 1536 : (grp[0] + nq) * 1536],
                        out_sb[:, 0 : nq * 1536],
                    )
    nc.compile()
    return nc


def _prep_inputs(coords, mask, pos, w1, b1, w2, b2):
    """Host-side input sharding + weight-only constant folding."""
    nan0 = np.isnan(coords[..., 0])
    c = np.nan_to_num(coords)
    vis = np.where(nan0, np.float32(0.0), mask).astype(np.float32)

    p_all = np.ascontiguousarray(vis.transpose(0, 2, 1)).reshape(BT, M)
    c_bt = np.ascontiguousarray(c.transpose(0, 2, 1, 3)).reshape(BT, M, 2)
    q_all = (p_all[:, :, None] * c_bt).reshape(BT, 2 * M).astype(np.float32)

    W2t = w2[:D_MOT]
    W2b = w2[D_MOT:]
    cvec = (b1 @ W2t + b2).astype(np.float32)             # (768,)
    v2 = (w1 @ W2t).astype(np.float32)                    # (512, 768)

    vh_dev = np.ascontiguousarray(
        v2.astype(np.float16).reshape(4, 128, E).transpose(1, 0, 2)
    ).reshape(128, 4 * E)
    cv_dev = np.ascontiguousarray(np.tile(cvec.astype(np.float16), R))[None, :]

    # Host-side correction only needed when pos_embed != 0 (the const is
    # then not rank-1); for this model pos_embed is always zeros.
    posw = pos.astype(np.float32) @ W2b                   # (M, 768)
    host_add = posw if np.any(posw) else None

    in_maps = []
    for i in range(N_CORES):
        rows = slice(i * R, (i + 1) * R)
        p_i = p_all[rows]                                 # (R, 256)
        q_i = q_all[rows]                                 # (R, 512)

        # L (512, 96): cols (j, r); j=0: P even rows, j=1: P odd rows, j=2: Q.
        la = np.zeros((512, 96), np.float32)
        la[0::2, 0:32] = p_i.T
        la[1::2, 32:64] = p_i.T
        la[:, 64:96] = q_i.T
        la_i = np.ascontiguousarray(
            la.astype(np.float16).reshape(4, 128, 96).transpose(1, 0, 2)
        ).reshape(128, 384)

        u0 = q_i[:, 0::2]
        u1 = q_i[:, 1::2]
        U = np.stack([u0, u1, -p_i, np.ones_like(p_i)], axis=0)  # (4, R, 256)
        ut_i = np.ascontiguousarray(U.astype(np.float16)).reshape(4, R * 256)

        in_maps.append({"la": la_i, "vh": vh_dev, "ut": ut_i, "cv": cv_dev})
    return in_maps, host_add


def _run(inputs, trace=False, trace_kwargs=None):
    from concourse.bass_utils import run_bass_kernel_spmd

    global _CACHED_NC
    if _CACHED_NC is None:
        _CACHED_NC = _build_nc()
    nc = _CACHED_NC

    coords = np.asarray(inputs["point_trajs_gt_coord"], dtype=np.float32)
    mask = np.asarray(inputs["point_trajs_visibility_mask"], dtype=np.float32)
    pos = np.asarray(inputs["pos_embed"], dtype=np.float32)
    w1 = np.asarray(inputs["fc1_w"], dtype=np.float32)
    b1 = np.asarray(inputs["fc1_b"], dtype=np.float32)
    w2 = np.asarray(inputs["fc_out_w"], dtype=np.float32)
    b2 = np.asarray(inputs["fc_out_b"], dtype=np.float32)

    in_maps, host_add = _prep_inputs(coords, mask, pos, w1, b1, w2, b2)
    res = run_bass_kernel_spmd(
        nc, in_maps, list(range(N_CORES)), trace=trace, **(trace_kwargs or {})
    )
    full = np.empty((BT, M, E), np.float32)
    for i in range(N_CORES):
        sh = np.asarray(res.results[i]["out"]).reshape(128, R, 2, E)
        full[i * R : (i + 1) * R] = sh.transpose(1, 2, 0, 3).reshape(R, M, E)
    if host_add is not None:
        full += host_add[None, :, :]
    return full.reshape(B, T, M, E), res


def kernel(**inputs):
    out, _ = _run(inputs, trace=False)
    return out


# revision 19
# speedup vs baseline: 1.0625x; 1.0625x over previous
"""Trainium2 Bass kernel for nn_CrossmotionModule (gnn_message_passing).

Reference computation (B=4, M=256, T=64, Dm=512, E=768):
    rel[b,m,t,n,k] = (c[b,m,t,k] - c[b,n,t,k]) * vis[b,m,t] * vis[b,n,t]
    fea[b,t,m,(n,k)] = rel                  # (B,T,M,512)
    h   = fea @ W1 + b1                     # (B,T,M,512)
    out = [h, pos] @ W2 + b2                # (B,T,M,768)

Algebraic collapse: with p = vis (BT,M), u0 = p*c0, u1 = p*c1 and the
host-folded fused weight V2 = W1 @ W2[:512] (512, 768):
    out[bt,m,e] = u0[m]*G0[e] + u1[m]*G1[e] - p[m]*G2[e] + const[m,e]
    G0[e] = sum_n p[n] V2[2n, e]
    G1[e] = sum_n p[n] V2[2n+1, e]
    G2[e] = sum_nk (p*c)[nk] V2[nk, e]
    const = b1 @ W2[:512] + pos @ W2[512:] + b2

The problem is HBM-write bound (192 MiB fp32 output) and, per-core, the
PE is output-streaming bound (the hardware throttles the PE to 1.2 GHz
under sustained load, ~0.83 ns per 128-lane PSUM column; measured 1280ns
per 1536-column row, back to back).  Design:

  * fp16 end-to-end (relative error ~4e-4, far under the 2e-2 gate):
    halves the output DMA traffic vs fp32 and needs no bf16-style split
    compensation anywhere.
  * const is rank-1 when pos == 0 (always true for this model): folded
    into the matmul as a 4th contraction row [u0, u1, -p, 1] x
    [G0; G1; G2; cvec] -- no per-row vector adds at all.  If pos != 0
    the (M,E) correction pos @ W2[512:] is added on the host.
  * G is computed on-device (L^T @ V2, fp32 PSUM), converted to fp16,
    and partition-regrouped (j*R+r, e) -> (j, r*E+e) through a DRAM
    bounce (matmul operands require contiguous 32-aligned partition
    ranges, so the regroup cannot be avoided; SBUF->SBUF regroup DMAs
    produce corrupt data on this toolchain).
  * vh arrives in 4 column chunks gating the 4 accumulation steps, so
    G matmuls start as soon as the first chunk lands.
  * Per row: 4 tiny-K (K=4) matmuls into two half-row PSUM tiles (2
    banks each, 2 generations = all 8 banks); fp32->fp16 PSUM->SBUF
    converts alternate DVE/Act; output DMAs drain a deep SBUF ring.

Sharding: data-parallel over bt = (b,t) flattened; 256 rows / 8 cores =
32 rows per core.  Weights replicated.  No cross-device communication.
"""

import numpy as np

B, M, T = 4, 256, 64
D_MOT, D_ABS, D_OUT = 512, 512, 768
N_CORES = 8
BT = B * T            # 256
R = BT // N_CORES     # 32 bt rows per core
E = D_OUT

_CACHED_NC = None


def _build_nc():
    """Build the SPMD Bass program (identical for all 8 cores)."""
    import concourse.bacc as bacc
    import concourse.bass as bass
    import concourse.mybir as mybir
    import concourse.tile as tile

    f32 = mybir.dt.float32
    f16 = mybir.dt.float16
    PSUM = bass.MemorySpace.PSUM

    nc = bacc.Bacc("TRN2", target_bir_lowering=False, debug=False)

    # Per-core inputs (host-prepared layouts; see _prep_inputs).
    la_d = nc.dram_tensor("la", [128, 4 * 96], f16, kind="ExternalInput")
    vh_d = nc.dram_tensor("vh", [128, 4 * E], f16, kind="ExternalInput")
    ut_d = nc.dram_tensor("ut", [4, R * 256], f16, kind="ExternalInput")
    cv_d = nc.dram_tensor("cv", [1, R * E], f16, kind="ExternalInput")
    out_d = nc.dram_tensor("out", [128, R * 2 * E], f16, kind="ExternalOutput")
    gscr_d = nc.dram_tensor("gscr", [3, R * E], f16)

    with tile.TileContext(nc) as tc:
        with tc.tile_pool(name="persist", bufs=1) as pers:
            ut_sb = pers.tile([4, R * 256], f16)
            g_sb = pers.tile([4, R * E], f16)

            # ---- prologue: G[(j,r), e] = L^T @ V2 ----
            with (
                tc.tile_pool(name="pro", bufs=1) as pro,
                tc.tile_pool(name="prop", bufs=1, space=PSUM) as prop,
            ):
                la_sb = pro.tile([128, 4 * 96], f16)
                vh_sb = pro.tile([128, 4 * E], f16)
                # la + vh chunk kk gate the kk-th accumulation step (region
                # deps); issue in critical-path order so later chunks arrive
                # while earlier accumulation steps run.
                nc.sync.dma_start(vh_sb[:, 0:E], vh_d[:, 0:E])
                nc.sync.dma_start(la_sb[:], la_d[:])
                for kk in range(1, 4):
                    nc.sync.dma_start(
                        vh_sb[:, kk * E : (kk + 1) * E],
                        vh_d[:, kk * E : (kk + 1) * E],
                    )
                nc.sync.dma_start(ut_sb[:], ut_d[:])
                nc.sync.dma_start(g_sb[3:4, :], cv_d[:])

                ghl_sb = pro.tile([96, E], f16)
                g_ps = prop.tile([96, E], f32)
                for kk in range(4):
                    for lo, hi in ((0, 512), (512, 768)):
                        nc.tensor.matmul(
                            g_ps[:, lo:hi],
                            la_sb[:, kk * 96 : (kk + 1) * 96],
                            vh_sb[:, kk * E + lo : kk * E + hi],
                            start=(kk == 0),
                            stop=(kk == 3),
                        )
                nc.scalar.copy(ghl_sb[:], g_ps[:])
                # Partition regroup (j*R+r, e)->(j, r*E+e) via DRAM bounce:
                # scatter on the write side, read back in 4 r-chunks so the
                # first main-loop matmuls only wait on chunk 0.
                nc.sync.dma_start(
                    gscr_d.rearrange("j (r e) -> (j r) e", r=R), ghl_sb[:]
                )
                CK = R // 4 * E
                for ck in range(4):
                    nc.sync.dma_start(
                        g_sb[0:3, ck * CK : (ck + 1) * CK],
                        gscr_d[:, ck * CK : (ck + 1) * CK],
                    )

            # ---- main loop: out[m, (r,w,e)] = U4_r^T G4_r ----
            groups = (
                [[0], [1]]
                + [list(range(g, g + 2)) for g in range(2, R - 4, 2)]
                + [[r] for r in range(R - 4, R)]
            )
            with (
                tc.tile_pool(name="mp", bufs=2, space=PSUM) as mp,
                tc.tile_pool(name="op", bufs=12) as op,
            ):
                ri = 0
                for grp in groups:
                    nq = len(grp)
                    out_sb = op.tile([128, nq * 1536], f16, tag="out_sb")
                    for q, r in enumerate(grp):
                        # Two half-row PSUM tiles (2 banks each) so the PE
                        # recycles banks at half-row grain and never stalls
                        # on a whole-row copy.
                        psL = mp.tile([128, 768], f32)
                        psR = mp.tile([128, 768], f32)
                        u0 = ut_sb[:, r * 256 : r * 256 + 128]
                        u1 = ut_sb[:, r * 256 + 128 : r * 256 + 256]
                        g0 = r * E
                        nc.tensor.matmul(psL[:, 0:512], u0, g_sb[:, g0 : g0 + 512])
                        nc.tensor.matmul(psL[:, 512:768], u0, g_sb[:, g0 + 512 : g0 + 768])
                        nc.tensor.matmul(psR[:, 0:512], u1, g_sb[:, g0 : g0 + 512])
                        nc.tensor.matmul(psR[:, 512:768], u1, g_sb[:, g0 + 512 : g0 + 768])
                        # GPSIMD cannot read PSUM; alternate DVE / Act.
                        for bi, ps in enumerate((psL, psR)):
                            dst = out_sb[
                                :, q * 1536 + bi * 768 : q * 1536 + (bi + 1) * 768
                            ]
                            if ri % 2 == 0:
                                nc.vector.tensor_copy(dst, ps[:])
                            else:
                                nc.scalar.copy(dst, ps[:])
                            ri += 1
                    nc.sync.dma_start(
                        out_d[:, grp[0] * 1536 : (grp[0] + nq) * 1536],
                        out_sb[:, 0 : nq * 1536],
                    )
    nc.compile()
    return nc


def _prep_inputs(coords, mask, pos, w1, b1, w2, b2):
    """Host-side input sharding + weight-only constant folding."""
    nan0 = np.isnan(coords[..., 0])
    c = np.nan_to_num(coords)
    vis = np.where(nan0, np.float32(0.0), mask).astype(np.float32)

    p_all = np.ascontiguousarray(vis.transpose(0, 2, 1)).reshape(BT, M)
    c_bt = np.ascontiguousarray(c.transpose(0, 2, 1, 3)).reshape(BT, M, 2)
    q_all = (p_all[:, :, None] * c_bt).reshape(BT, 2 * M).astype(np.float32)

    W2t = w2[:D_MOT]
    W2b = w2[D_MOT:]
    cvec = (b1 @ W2t + b2).astype(np.float32)             # (768,)
    v2 = (w1 @ W2t).astype(np.float32)                    # (512, 768)

    vh_dev = np.ascontiguousarray(
        v2.astype(np.float16).reshape(4, 128, E).transpose(1, 0, 2)
    ).reshape(128, 4 * E)
    cv_dev = np.ascontiguousarray(np.tile(cvec.astype(np.float16), R))[None, :]

    # Host-side correction only needed when pos_embed != 0 (the const is
    # then not rank-1); for this model pos_embed is always zeros.
    posw = pos.astype(np.float32) @ W2b                   # (M, 768)
    host_add = posw if np.any(posw) else None

    in_maps = []
    for i in range(N_CORES):
        rows = slice(i * R, (i + 1) * R)
        p_i = p_all[rows]                                 # (R, 256)
        q_i = q_all[rows]                                 # (R, 512)

        # L (512, 96): cols (j, r); j=0: P even rows, j=1: P odd rows, j=2: Q.
        la = np.zeros((512, 96), np.float32)
        la[0::2, 0:32] = p_i.T
        la[1::2, 32:64] = p_i.T
        la[:, 64:96] = q_i.T
        la_i = np.ascontiguousarray(
            la.astype(np.float16).reshape(4, 128, 96).transpose(1, 0, 2)
        ).reshape(128, 384)

        u0 = q_i[:, 0::2]
        u1 = q_i[:, 1::2]
        U = np.stack([u0, u1, -p_i, np.ones_like(p_i)], axis=0)  # (4, R, 256)
        ut_i = np.ascontiguousarray(U.astype(np.float16)).reshape(4, R * 256)

        in_maps.append({"la": la_i, "vh": vh_dev, "ut": ut_i, "cv": cv_dev})
    return in_maps, host_add


def _run(inputs, trace=False, trace_kwargs=None):
    from concourse.bass_utils import run_bass_kernel_spmd

    global _CACHED_NC
    if _CACHED_NC is None:
        _CACHED_NC = _build_nc()
    nc = _CACHED_NC

    coords = np.asarray(inputs["point_trajs_gt_coord"], dtype=np.float32)
    mask = np.asarray(inputs["point_trajs_visibility_mask"], dtype=np.float32)
    pos = np.asarray(inputs["pos_embed"], dtype=np.float32)
    w1 = np.asarray(inputs["fc1_w"], dtype=np.float32)
    b1 = np.asarray(inputs["fc1_b"], dtype=np.float32)
    w2 = np.asarray(inputs["fc_out_w"], dtype=np.float32)
    b2 = np.asarray(inputs["fc_out_b"], dtype=np.float32)

    in_maps, host_add = _prep_inputs(coords, mask, pos, w1, b1, w2, b2)
    res = run_bass_kernel_spmd(
        nc, in_maps, list(range(N_CORES)), trace=trace, **(trace_kwargs or {})
    )
    full = np.empty((BT, M, E), np.float32)
    for i in range(N_CORES):
        sh = np.asarray(res.results[i]["out"]).reshape(128, R, 2, E)
        full[i * R : (i + 1) * R] = sh.transpose(1, 2, 0, 3).reshape(R, M, E)
    if host_add is not None:
        full += host_add[None, :, :]
    return full.reshape(B, T, M, E), res


def kernel(**inputs):
    out, _ = _run(inputs, trace=False)
    return out


# revision 22
# speedup vs baseline: 1.0796x; 1.0162x over previous
"""Trainium2 Bass kernel for nn_CrossmotionModule (gnn_message_passing).

Reference computation (B=4, M=256, T=64, Dm=512, E=768):
    rel[b,m,t,n,k] = (c[b,m,t,k] - c[b,n,t,k]) * vis[b,m,t] * vis[b,n,t]
    fea[b,t,m,(n,k)] = rel                  # (B,T,M,512)
    h   = fea @ W1 + b1                     # (B,T,M,512)
    out = [h, pos] @ W2 + b2                # (B,T,M,768)

Algebraic collapse: with p = vis (BT,M), u0 = p*c0, u1 = p*c1 and the
host-folded fused weight V2 = W1 @ W2[:512] (512, 768):
    out[bt,m,e] = u0[m]*G0[e] + u1[m]*G1[e] - p[m]*G2[e] + const[m,e]
    G0[e] = sum_n p[n] V2[2n, e]
    G1[e] = sum_n p[n] V2[2n+1, e]
    G2[e] = sum_nk (p*c)[nk] V2[nk, e]
    const = b1 @ W2[:512] + pos @ W2[512:] + b2

The problem is HBM-write bound (192 MiB fp32 output) and, per-core, the
PE is output-streaming bound (the hardware throttles the PE to 1.2 GHz
under sustained load, ~0.83 ns per 128-lane PSUM column; measured 1280ns
per 1536-column row, back to back).  Design:

  * fp16 end-to-end (relative error ~4e-4, far under the 2e-2 gate):
    halves the output DMA traffic vs fp32 and needs no bf16-style split
    compensation anywhere.
  * const is rank-1 when pos == 0 (always true for this model): folded
    into the matmul as a 4th contraction row [u0, u1, -p, 1] x
    [G0; G1; G2; cvec] -- no per-row vector adds at all.  If pos != 0
    the (M,E) correction pos @ W2[512:] is added on the host.
  * G is computed on-device (L^T @ V2, fp32 PSUM), converted to fp16,
    and partition-regrouped (j*R+r, e) -> (j, r*E+e) through a DRAM
    bounce (matmul operands require contiguous 32-aligned partition
    ranges, so the regroup cannot be avoided; SBUF->SBUF regroup DMAs
    produce corrupt data on this toolchain).
  * vh arrives in 4 column chunks gating the 4 accumulation steps, so
    G matmuls start as soon as the first chunk lands.
  * Per row: 4 tiny-K (K=4) matmuls into two half-row PSUM tiles (2
    banks each, 2 generations = all 8 banks); fp32->fp16 PSUM->SBUF
    converts alternate DVE/Act; output DMAs drain a deep SBUF ring.

Sharding: data-parallel over bt = (b,t) flattened; 256 rows / 8 cores =
32 rows per core.  Weights replicated.  No cross-device communication.
"""

import numpy as np

B, M, T = 4, 256, 64
D_MOT, D_ABS, D_OUT = 512, 512, 768
N_CORES = 8
BT = B * T            # 256
R = BT // N_CORES     # 32 bt rows per core
E = D_OUT

_CACHED_NC = None


def _build_nc():
    """Build the SPMD Bass program (identical for all 8 cores)."""
    import concourse.bacc as bacc
    import concourse.bass as bass
    import concourse.mybir as mybir
    import concourse.tile as tile

    f32 = mybir.dt.float32
    f16 = mybir.dt.float16
    PSUM = bass.MemorySpace.PSUM

    nc = bacc.Bacc("TRN2", target_bir_lowering=False, debug=False)

    # Per-core inputs (host-prepared layouts; see _prep_inputs).
    la_d = nc.dram_tensor("la", [128, 4 * 96], f16, kind="ExternalInput")
    vh_d = nc.dram_tensor("vh", [128, 4 * E], f16, kind="ExternalInput")
    ut_d = nc.dram_tensor("ut", [4, R * 256], f16, kind="ExternalInput")
    cv_d = nc.dram_tensor("cv", [1, R * E], f16, kind="ExternalInput")
    out_d = nc.dram_tensor("out", [128, R * 2 * E], f16, kind="ExternalOutput")
    gscr_d = nc.dram_tensor("gscr", [3, R * E], f16)

    with tile.TileContext(nc) as tc:
        with tc.tile_pool(name="persist", bufs=1) as pers:
            ut_sb = pers.tile([4, R * 256], f16)
            g_sb = pers.tile([4, R * E], f16)

            # ---- prologue: G[(j,r), e] = L^T @ V2 ----
            with (
                tc.tile_pool(name="pro", bufs=1) as pro,
                tc.tile_pool(name="prop", bufs=1, space=PSUM) as prop,
            ):
                la_sb = pro.tile([128, 4 * 96], f16)
                vh_sb = pro.tile([128, 4 * E], f16)
                # la + vh chunk kk gate the kk-th accumulation step (region
                # deps); issue in critical-path order so later chunks arrive
                # while earlier accumulation steps run.
                nc.sync.dma_start(vh_sb[:, 0:E], vh_d[:, 0:E])
                nc.sync.dma_start(la_sb[:], la_d[:])
                for kk in range(1, 4):
                    nc.sync.dma_start(
                        vh_sb[:, kk * E : (kk + 1) * E],
                        vh_d[:, kk * E : (kk + 1) * E],
                    )
                nc.sync.dma_start(ut_sb[:], ut_d[:])
                nc.sync.dma_start(g_sb[3:4, :], cv_d[:])

                ghl_sb = pro.tile([96, E], f16)
                g_ps = prop.tile([96, E], f32)
                for kk in range(4):
                    for lo, hi in ((0, 512), (512, 768)):
                        nc.tensor.matmul(
                            g_ps[:, lo:hi],
                            la_sb[:, kk * 96 : (kk + 1) * 96],
                            vh_sb[:, kk * E + lo : kk * E + hi],
                            start=(kk == 0),
                            stop=(kk == 3),
                        )
                nc.scalar.copy(ghl_sb[:], g_ps[:])
                # Partition regroup (j*R+r, e)->(j, r*E+e) via DRAM bounce:
                # scatter on the write side, read back in 4 r-chunks so the
                # first main-loop matmuls only wait on chunk 0.
                nc.sync.dma_start(
                    gscr_d.rearrange("j (r e) -> (j r) e", r=R), ghl_sb[:]
                )
                CK = R // 4 * E
                for ck in range(4):
                    nc.sync.dma_start(
                        g_sb[0:3, ck * CK : (ck + 1) * CK],
                        gscr_d[:, ck * CK : (ck + 1) * CK],
                    )

            # ---- main loop: out[m, (r,w,e)] = U4_r^T G4_r ----
            groups = (
                [[0], [1]]
                + [list(range(g, g + 4)) for g in range(2, 26, 4)]
                + [[26, 27], [28, 29], [30], [31]]
            )
            with (
                tc.tile_pool(name="mp", bufs=2, space=PSUM) as mp,
                tc.tile_pool(name="op", bufs=11) as op,
            ):
                ri = 0
                for grp in groups:
                    nq = len(grp)
                    out_sb = op.tile([128, nq * 1536], f16, tag="out_sb")
                    for q, r in enumerate(grp):
                        # Two half-row PSUM tiles (2 banks each) so the PE
                        # recycles banks at half-row grain and never stalls
                        # on a whole-row copy.
                        psL = mp.tile([128, 768], f32)
                        psR = mp.tile([128, 768], f32)
                        u0 = ut_sb[:, r * 256 : r * 256 + 128]
                        u1 = ut_sb[:, r * 256 + 128 : r * 256 + 256]
                        g0 = r * E
                        nc.tensor.matmul(psL[:, 0:512], u0, g_sb[:, g0 : g0 + 512])
                        nc.tensor.matmul(psL[:, 512:768], u0, g_sb[:, g0 + 512 : g0 + 768])
                        nc.tensor.matmul(psR[:, 0:512], u1, g_sb[:, g0 : g0 + 512])
                        nc.tensor.matmul(psR[:, 512:768], u1, g_sb[:, g0 + 512 : g0 + 768])
                        # GPSIMD cannot read PSUM; alternate DVE / Act.
                        for bi, ps in enumerate((psL, psR)):
                            dst = out_sb[
                                :, q * 1536 + bi * 768 : q * 1536 + (bi + 1) * 768
                            ]
                            if ri % 2 == 0:
                                nc.vector.tensor_copy(dst, ps[:])
                            else:
                                nc.scalar.copy(dst, ps[:])
                            ri += 1
                    if grp == [R - 1]:
                        # Split the final row into two half-row DMAs so the
                        # last exposed transfer is short.
                        g1 = (R - 1) * 1536
                        nc.sync.dma_start(out_d[:, g1 : g1 + 768], out_sb[:, 0:768])
                        nc.sync.dma_start(
                            out_d[:, g1 + 768 : g1 + 1536], out_sb[:, 768:1536]
                        )
                    else:
                        nc.sync.dma_start(
                            out_d[:, grp[0] * 1536 : (grp[0] + nq) * 1536],
                            out_sb[:, 0 : nq * 1536],
                        )
    nc.compile()
    return nc


def _prep_inputs(coords, mask, pos, w1, b1, w2, b2):
    """Host-side input sharding + weight-only constant folding."""
    nan0 = np.isnan(coords[..., 0])
    c = np.nan_to_num(coords)
    vis = np.where(nan0, np.float32(0.0), mask).astype(np.float32)

    p_all = np.ascontiguousarray(vis.transpose(0, 2, 1)).reshape(BT, M)
    c_bt = np.ascontiguousarray(c.transpose(0, 2, 1, 3)).reshape(BT, M, 2)
    q_all = (p_all[:, :, None] * c_bt).reshape(BT, 2 * M).astype(np.float32)

    W2t = w2[:D_MOT]
    W2b = w2[D_MOT:]
    cvec = (b1 @ W2t + b2).astype(np.float32)             # (768,)
    v2 = (w1 @ W2t).astype(np.float32)                    # (512, 768)

    vh_dev = np.ascontiguousarray(
        v2.astype(np.float16).reshape(4, 128, E).transpose(1, 0, 2)
    ).reshape(128, 4 * E)
    cv_dev = np.ascontiguousarray(np.tile(cvec.astype(np.float16), R))[None, :]

    # Host-side correction only needed when pos_embed != 0 (the const is
    # then not rank-1); for this model pos_embed is always zeros.
    posw = pos.astype(np.float32) @ W2b                   # (M, 768)
    host_add = posw if np.any(posw) else None

    in_maps = []
    for i in range(N_CORES):
        rows = slice(i * R, (i + 1) * R)
        p_i = p_all[rows]                                 # (R, 256)
        q_i = q_all[rows]                                 # (R, 512)

        # L (512, 96): cols (j, r); j=0: P even rows, j=1: P odd rows, j=2: Q.
        la = np.zeros((512, 96), np.float32)
        la[0::2, 0:32] = p_i.T
        la[1::2, 32:64] = p_i.T
        la[:, 64:96] = q_i.T
        la_i = np.ascontiguousarray(
            la.astype(np.float16).reshape(4, 128, 96).transpose(1, 0, 2)
        ).reshape(128, 384)

        u0 = q_i[:, 0::2]
        u1 = q_i[:, 1::2]
        U = np.stack([u0, u1, -p_i, np.ones_like(p_i)], axis=0)  # (4, R, 256)
        ut_i = np.ascontiguousarray(U.astype(np.float16)).reshape(4, R * 256)

        in_maps.append({"la": la_i, "vh": vh_dev, "ut": ut_i, "cv": cv_dev})
    return in_maps, host_add


def _run(inputs, trace=False, trace_kwargs=None):
    from concourse.bass_utils import run_bass_kernel_spmd

    global _CACHED_NC
    if _CACHED_NC is None:
        _CACHED_NC = _build_nc()
    nc = _CACHED_NC

    coords = np.asarray(inputs["point_trajs_gt_coord"], dtype=np.float32)
    mask = np.asarray(inputs["point_trajs_visibility_mask"], dtype=np.float32)
    pos = np.asarray(inputs["pos_embed"], dtype=np.float32)
    w1 = np.asarray(inputs["fc1_w"], dtype=np.float32)
    b1 = np.asarray(inputs["fc1_b"], dtype=np.float32)
    w2 = np.asarray(inputs["fc_out_w"], dtype=np.float32)
    b2 = np.asarray(inputs["fc_out_b"], dtype=np.float32)

    in_maps, host_add = _prep_inputs(coords, mask, pos, w1, b1, w2, b2)
    res = run_bass_kernel_spmd(
        nc, in_maps, list(range(N_CORES)), trace=trace, **(trace_kwargs or {})
    )
    full = np.empty((BT, M, E), np.float32)
    for i in range(N_CORES):
        sh = np.asarray(res.results[i]["out"]).reshape(128, R, 2, E)
        full[i * R : (i + 1) * R] = sh.transpose(1, 2, 0, 3).reshape(R, M, E)
    if host_add is not None:
        full += host_add[None, :, :]
    return full.reshape(B, T, M, E), res


def kernel(**inputs):
    out, _ = _run(inputs, trace=False)
    return out
